# revision 46
# baseline (speedup 1.0000x reference)
"""BCOP (block-convolution orthogonal parameterization) forward on 8 TRN2 cores.

v3: factored conv via the BCOP projection structure (fewer PE cycles than the
direct 3x3), bf16 data path, fp32r Bjorck (bf16 Bjorck fails numerically).

Math (validated vs reference in fp64 numpy, rel err 8e-7):
  - sigma via repeated squaring (as before), W0 = A/s, 20 fp32r Bjorck iters
    maintaining W and WT.
  - conv weight w[co,ci,kh,kw] = (H @ p3[kw,kh])[ci,co] with
    p3 = b1 (x) b2 (2x2 matrix-conv), b's built from projections
    Q0..Q3 = (masked ortho rows) -> out = sum p3[kw,kh]^T (H^T x)_shift.
    Factoring each 2x2 stage through the projections:
      z  = H^T x + bias           (bias cancels in every difference term and
                                   rides the passthrough chain to the output)
      y  = (Q1 Q0).Dz + Q1.Az + Q0.d(r'+1) + z(r'+1,c'+1)   on a 65x65 grid
      out= (Q3 Q2).DD + Q3.Ay + Q2.D'(r) + y(r,c)           on the 64x64 grid
    where d/Dz/Az/D'/DD/Ay are spatial difference arrays of z resp. y.
    That is 3 matrix applies + 1 passthrough per stage instead of 4 applies:
    PE cycles per image 117k vs 147k direct; passthrough adds are fused into
    the PSUM->SBUF evacuations on DVE.
  - lhsT operands: T10 = Q0@Q1 (= (Q1 Q0)^T), T32 = Q2@Q3, PQ_i themselves
    (symmetric), H_sb = PE-transpose of the gathered WT[0].

Parallelization: per-matrix construction sharded across cores (core i gets
matrix i mod 5), bf16 AllGather of the WT halves, assembly + conv replicated /
data-parallel over batch (4 images per core).

Engine budget per core: PE ~470k cycles (HAM-capped ~1.9GHz -> ~256us);
DVE does diffs + evac-adds (~160us); scalar does z evacs (+bias) and small
copies; GpSimd does the x fp32->bf16 cast + circular pads (front-loaded).

Image pipeline: PE emission order z(0), [s1(b), z(b+1), s2(b)] so the vector
diff batches of one stage hide under the next stage's matmuls.
"""

import numpy as np

import concourse.bass as bass
import concourse.mybir as mybir
import concourse.tile as tile
from concourse import bacc
from concourse.bass_utils import run_bass_kernel_spmd

P = 128
C = 256
NK = 5
N_CORES = 8
B_TOTAL = 32
B_CORE = B_TOTAL // N_CORES
H = 64
PH = 66
NPIX = H * H
BJORCK_ITERS = 20

F32 = mybir.dt.float32
F32R = mybir.dt.float32r
BF16 = mybir.dt.bfloat16
ALU = mybir.AluOpType
ACTF = mybir.ActivationFunctionType


def build_body(tc, out_ap, xs, pmk, u0k, bias_ap, ctx):
    nc = tc.nc
    from concourse.masks import make_identity

    persist = ctx.enter_context(tc.tile_pool(name="persist", bufs=1))
    small = ctx.enter_context(tc.tile_pool(name="small", bufs=3))
    xpool = ctx.enter_context(tc.tile_pool(name="xpool", bufs=4))
    xstgp = ctx.enter_context(tc.tile_pool(name="xstgp", bufs=2))

    # ---- tiny input DMAs first (not stuck behind the big x loads) ----
    U_sb = persist.tile([P, 2, 1], F32R)
    for tr in range(2):
        nc.sync.dma_start(U_sb[:, tr, :], u0k[tr * P:(tr + 1) * P, :])
    bias_sb = persist.tile([P, 2, 1], F32)
    for mt in range(2):
        nc.sync.dma_start(bias_sb[:, mt, :], bias_ap[mt * P:(mt + 1) * P].unsqueeze(1))
    pm_sb = persist.tile([P, 2, C], F32R)
    for tr in range(2):
        nc.sync.dma_start(pm_sb[:, tr, :], pmk[tr * P:(tr + 1) * P, :])

    # ---- constants FIRST: make_identity's gpsimd op must precede the long
    # x-prep cast chain on the gpsimd queue, or everything scalar that
    # depends on ID1 stalls ~40us into the construction ----
    ID1 = persist.tile([P, P], F32)
    make_identity(nc, ID1)
    ID16 = persist.tile([P, P], BF16)
    nc.scalar.mul(ID16[:], ID1[:], 1.0)
    I15 = persist.tile([P, 2, C], F32)
    nc.vector.memset(I15[:], 0.0)
    ones_r = small.tile([1, P], F32, tag="ones")
    nc.vector.memset(ones_r[:], 1.0)
    for mt in range(2):
        nc.scalar.mul(I15[:, mt, mt * P:(mt + 1) * P], ID1[:], 1.5)

    # ---- x: DMA fp32 staging -> GpSimd casts to bf16 + circular pads ----
    xp = []
    for b in range(B_CORE):
        xp.append([])
        for tr in range(2):
            xpt = xpool.tile([P, PH, PH], BF16, tag="xp", name=f"xp_{b}_{tr}")
            xp[b].append(xpt)

    def emit_xprep(b):
        for tr in range(2):
            xpt = xp[b][tr]
            # stage + cast in quarter-image chunks to keep SBUF staging small
            for hh in range(4):
                xst = xstgp.tile([P, H // 4, H], F32, tag="xst",
                                 name=f"xst_{b}_{tr}_{hh}")
                nc.sync.dma_start(xst[:], xs[b, tr * P:(tr + 1) * P,
                                              hh * 16:(hh + 1) * 16, :])
                nc.gpsimd.tensor_copy(
                    xpt[:, 1 + hh * 16:1 + (hh + 1) * 16, 1:65], xst[:])
            nc.gpsimd.tensor_copy(xpt[:, 1:65, 0:1], xpt[:, 1:65, 64:65])
            nc.gpsimd.tensor_copy(xpt[:, 1:65, 65:66], xpt[:, 1:65, 1:2])
            nc.gpsimd.tensor_copy(xpt[:, 0:1, :], xpt[:, 64:65, :])
            nc.gpsimd.tensor_copy(xpt[:, 65:66, :], xpt[:, 1:2, :])

    for b in range(B_CORE):
        emit_xprep(b)

    RB = persist.tile([P, 1], F32)             # broadcast 1/s (own matrix)
    WTf16 = persist.tile([P, NK, 2, C], BF16)  # gathered ortho^T, all matrices
    PQ_sb = persist.tile([P, 4, 2, C], BF16)   # Q0..Q3 projections
    T10_sb = persist.tile([P, 2, C], BF16)     # Q0@Q1  (lhsT for Q1Q0-apply)
    T32_sb = persist.tile([P, 2, C], BF16)     # Q2@Q3  (lhsT for Q3Q2-apply)
    H_sb = persist.tile([P, 2, C], BF16)       # H[ci,co] (lhsT for H^T-apply)

    def flat(ap3):
        return ap3.rearrange("p a b -> p (a b)")

    def prod_mms(out_ps, X3, Y3, n_tr=2):
        """dst[mt] += X[tr][:, mt]^T @ Y[tr]; ONE accumulation group per bank."""
        first = True
        for mt in range(2):
            for tr in range(n_tr):
                last = (mt == 1 and tr == n_tr - 1)
                nc.tensor.matmul(out_ps[:, mt * C:(mt + 1) * C],
                                 X3[:, tr, mt * P:(mt + 1) * P], Y3[:, tr, :],
                                 start=first, stop=last)
                first = False

    with tc.tile_pool(name="build", bufs=1) as build, \
         tc.tile_pool(name="wstate", bufs=2) as wpool, \
         tc.tile_pool(name="mpool", bufs=4) as mpool, \
         tc.tile_pool(name="vpool", bufs=8) as vpool, \
         tc.tile_pool(name="ccdram", bufs=1, space="DRAM") as ccdram:

        Wcur = wpool.tile([P, 2, C], F32R, tag="W")
        WTcur = wpool.tile([P, 2, C], F32R, tag="WT")
        G_sb = build.tile([P, 2, C], F32R)
        G2_sb = build.tile([P, 2, C], F32R)
        G16_sb = build.tile([P, 2, C], F32R)

        # ============ phase 1: sigma via repeated squaring (own matrix) ======
        with tc.tile_pool(name="ps1", bufs=2, space="PSUM") as ps1:
            def kw1(nm):
                """p-state keep-warm (see phase 2)."""
                for k in range(2):
                    wps_ = ps1.tile([P, P], F32, tag="warm1", bufs=1,
                                    name=f"kw1_{nm}_{k}", padded_shape=[P, 512])
                    nc.tensor.matmul(wps_[:], ID16[:], ID16[:],
                                     start=True, stop=True)

            def sq_prod(prev, dst, nm):
                """split-bank 256x256 product; the two half evacuations drain
                on scalar and vector in parallel while keep-warm matmuls
                bridge the PE to the next squaring."""
                halves = []
                for mt in range(2):
                    h = ps1.tile([P, C], F32, tag=f"sq{mt}", bufs=1,
                                 name=f"{nm}_{mt}", padded_shape=[P, 512])
                    for tr in range(2):
                        nc.tensor.matmul(h[:],
                                         prev[:, tr, mt * P:(mt + 1) * P],
                                         prev[:, tr, :],
                                         start=(tr == 0), stop=(tr == 1))
                    halves.append(h)
                kw1(nm)
                nc.scalar.copy(dst[:, 0, :], halves[0][:])
                nc.vector.tensor_copy(dst[:, 1, :], halves[1][:])

            def matvec(G3, vin, nm):
                vout = vpool.tile([P, 2], F32R, tag="v", name=f"v_{nm}")
                for mt in range(2):
                    vps = ps1.tile([P, 1], F32, tag="vps", bufs=2,
                                   name=f"vp_{nm}_{mt}")
                    for tr in range(2):
                        nc.tensor.matmul(
                            vps[:], G3[:, tr, mt * P:(mt + 1) * P].bitcast(F32),
                            vin[:, tr:tr + 1].bitcast(F32),
                            start=(tr == 0), stop=(tr == 1))
                    nc.scalar.copy(vout[:, mt:mt + 1], vps[:])
                return vout

            sq_prod(pm_sb, G_sb, "g1")
            # interleave the first matvec behind the G matmuls
            v1 = matvec(pm_sb, U_sb, "v1")

            prev = G_sb
            mvs = {}
            for pw in (2, 4, 8, 16):
                if pw == 2:
                    dst = G2_sb
                elif pw == 16:
                    dst = G16_sb
                else:
                    dst = build.tile([P, 2, C], F32R, tag="gtmp",
                                     name=f"g{pw}", bufs=2)
                sq_prod(prev, dst, f"sq{pw}")
                if pw == 4:
                    mvs["m1"] = matvec(G2_sb, v1, "m1")
                prev = dst

            m1 = mvs["m1"]
            m2 = matvec(G16_sb, m1, "m2")
            m3 = matvec(G_sb, m2, "m3")

            def dot(va, vb, nm):
                dps = ps1.tile([1, 1], F32, tag="vps", bufs=2, name=f"d_{nm}")
                for tr in range(2):
                    nc.tensor.matmul(dps[:], va[:, tr:tr + 1].bitcast(F32),
                                     vb[:, tr:tr + 1].bitcast(F32),
                                     start=(tr == 0), stop=(tr == 1))
                return dps

            dps0 = dot(v1, m2, "0")
            dps1 = dot(v1, m3, "1")
            dsb = small.tile([1, 3], F32, tag="dsb")
            nc.vector.tensor_copy(dsb[:, 0:1], dps0[:])
            nc.vector.reciprocal(dsb[:, 1:2], dps1[:])
            nc.vector.tensor_mul(dsb[:, 2:3], dsb[:, 0:1], dsb[:, 1:2])
            rsb = small.tile([1, 1], F32, tag="rsb")
            nc.scalar.sqrt(rsb[:], dsb[:, 2:3])
            # broadcast 1/s across partitions via PE (gpsimd is busy with x)
            rps = ps1.tile([P, 1], F32, tag="vps", bufs=2, name="rps")
            nc.tensor.matmul(rps[:], ones_r[:], rsb[:], start=True, stop=True)
            nc.scalar.copy(RB[:], rps[:])
            # W0 = A * r, then WT0 = W0^T via PE transpose
            nc.vector.tensor_scalar_mul(Wcur[:], pm_sb[:], RB[:, 0:1])
            for tr in range(2):
                for mt in range(2):
                    tps = ps1.tile([P, P], F32, tag="tp")
                    nc.tensor.transpose(
                        tps[:], Wcur[:, tr, mt * P:(mt + 1) * P].bitcast(F32),
                        ID1[:])
                    nc.scalar.copy(WTcur[:, mt, tr * P:(tr + 1) * P], tps[:])

        # ================= phase 2: Bjorck (own matrix, fp32r) ===============
        # Split-half PSUM groups: the two output row-tiles of each 256x256
        # product accumulate in separate banks so the first half's evacuation
        # (and the M half it feeds) overlaps the second half's matmuls.
        with tc.tile_pool(name="ps2", bufs=2, space="PSUM") as ps2:
            def keep_warm(it, slot):
                """No-op matmuls placed in the iteration's dependency stalls:
                the PE p-state (0.65/1.2/2.4GHz) ramps only under continuous
                execution, and the serial Bjorck chain otherwise idles the PE
                a few hundred ns at every cross-engine hop, parking it at the
                low clock."""
                for k in range(2):
                    wps_ = ps2.tile([P, P], F32, tag="warm", bufs=2,
                                    name=f"warm_{it}_{slot}_{k}",
                                    padded_shape=[P, 512])
                    nc.tensor.matmul(wps_[:], ID16[:], ID16[:],
                                     start=True, stop=True)

            for it in range(BJORCK_ITERS):
                last = it == BJORCK_ITERS - 1
                Wnxt = None if last else wpool.tile([P, 2, C], F32R, tag="W",
                                                    name=f"W_{it}")
                WTnxt = wpool.tile([P, 2, C], F32R, tag="WT", name=f"WT_{it}")
                gps = []
                for mt in range(2):
                    g = ps2.tile([P, C], F32, tag=f"g{mt}", bufs=1,
                                 name=f"g{mt}_{it}", padded_shape=[P, 512])
                    for tr in range(2):
                        nc.tensor.matmul(g[:], Wcur[:, tr, mt * P:(mt + 1) * P],
                                         Wcur[:, tr, :],
                                         start=(tr == 0), stop=(tr == 1))
                    gps.append(g)
                keep_warm(it, 0)
                m_sb = mpool.tile([P, 2, C], F32R, tag="m", name=f"m_{it}")
                for mt in range(2):
                    nc.vector.scalar_tensor_tensor(
                        m_sb[:, mt, :], gps[mt][:], -0.5, I15[:, mt, :],
                        op0=ALU.mult, op1=ALU.add)
                wps, wtps = [], []
                if not last:
                    wps = [ps2.tile([P, C], F32, tag=f"w{mt}", bufs=1,
                                    name=f"w{mt}_{it}", padded_shape=[P, 512])
                           for mt in range(2)]
                wtps = [ps2.tile([P, C], F32, tag=f"x{mt}", bufs=1,
                                 name=f"x{mt}_{it}", padded_shape=[P, 512])
                        for mt in range(2)]
                for tr in range(2):
                    for mt in range(2):
                        if not last:
                            nc.tensor.matmul(
                                wps[mt][:], WTcur[:, tr, mt * P:(mt + 1) * P],
                                m_sb[:, tr, :], start=(tr == 0), stop=(tr == 1))
                        nc.tensor.matmul(
                            wtps[mt][:], m_sb[:, tr, mt * P:(mt + 1) * P],
                            WTcur[:, tr, :], start=(tr == 0), stop=(tr == 1))
                keep_warm(it, 1)
                for mt in range(2):
                    if not last:
                        nc.scalar.copy(Wnxt[:, mt, :], wps[mt][:])
                    nc.vector.tensor_copy(WTnxt[:, mt, :], wtps[mt][:])
                if Wnxt is not None:
                    Wcur = Wnxt
                WTcur = WTnxt

        # ============ AllGather the needed WT halves across cores (bf16) =====
        # Cores 0-4 contribute their tr=0 half; core 5 (a k=0 duplicate)
        # contributes k=0's tr=1 half via a partition-id-predicated DMA.
        cc16 = build.tile([P, 2, C], BF16)
        for tr in range(2):
            nc.vector.tensor_copy(cc16[:, tr, :], WTcur[:, tr, :])
        cc_in = ccdram.tile([1, P * C], BF16)
        cc_out = ccdram.tile([N_CORES, P * C], BF16, addr_space="Shared")
        pid = nc.sync.partition_id()
        nc.sync.dma_start(cc_in[0].rearrange("(p n) -> p n", p=P),
                          cc16[:, 0, :], cond=(pid != 5))
        nc.sync.dma_start(cc_in[0].rearrange("(p n) -> p n", p=P),
                          cc16[:, 1, :], cond=(pid == 5))
        nc.gpsimd.collective_compute(
            "AllGather", ALU.bypass, ins=[cc_in.opt()], outs=[cc_out.opt()],
            replica_groups=[list(range(N_CORES))])
        for k in range(NK):
            nc.sync.dma_start(WTf16[:, k, 0, :],
                              cc_out[k].rearrange("(p n) -> p n", p=P))
        nc.sync.dma_start(WTf16[:, 0, 1, :],
                          cc_out[NK].rearrange("(p n) -> p n", p=P))

        # ============ phase 3: tiny assembly: PQ, T10, T32, H ============
        with tc.tile_pool(name="ps3", bufs=4, space="PSUM") as ps3:
            # Keep-warm matmuls spanning the AllGather dead window (~25us):
            # they depend only on local data, so they run while the collective
            # flows and the PE enters the assembly + z(0) burst at full clock
            # instead of restarting from the 0.65GHz p-state.
            for k in range(56):
                wps_ = ps3.tile([P, 512], F32, tag="warm3", bufs=2,
                                name=f"kw3_{k}")
                nc.tensor.matmul(wps_[:], ID16[:], flat(cc16),
                                 start=True, stop=True)
            # H_sb[ci-part(tile mt), co-free(tile tr)] = transpose of WT0
            for tr in range(2):
                for mt in range(2):
                    tps = ps3.tile([P, P], BF16, tag="tp", name=f"ht{tr}{mt}",
                                   bufs=2, padded_shape=[P, 1024])
                    nc.tensor.transpose(tps[:], WTf16[:, 0, tr, mt * P:(mt + 1) * P],
                                        ID16[:])
                    nc.scalar.copy(H_sb[:, mt, tr * P:(tr + 1) * P], tps[:])
            for i in range(4):
                qps = ps3.tile([P, 2 * C], F32, tag="as", name=f"q_{i}", bufs=2)
                prod_mms(qps, WTf16[:, i + 1], WTf16[:, i + 1], n_tr=1)
                nc.scalar.copy(flat(PQ_sb[:, i]), qps[:])
            t10 = ps3.tile([P, 2 * C], F32, tag="as", name="t10", bufs=2)
            prod_mms(t10, PQ_sb[:, 0], PQ_sb[:, 1])
            nc.scalar.copy(flat(T10_sb), t10[:])
            t32 = ps3.tile([P, 2 * C], F32, tag="as", name="t32", bufs=2)
            prod_mms(t32, PQ_sb[:, 2], PQ_sb[:, 3])
            nc.scalar.copy(flat(T32_sb), t32[:])

    # ================= phase 4: factored conv (bf16) =================
    # grids: z on 66x66 (same geometry as xp), y on 65x65, out on 64x64.
    NZ = PH * PH            # 4356
    NY = 65 * 65            # 4225
    ZBLK = [(s * 512, min(512, NZ - s * 512)) for s in range((NZ + 511) // 512)]
    YROW = [(r0, min(7, 65 - r0)) for r0 in range(0, 65, 7)]

    with tc.tile_pool(name="zpool", bufs=1) as zpool, \
         tc.tile_pool(name="dpool", bufs=1) as dpool, \
         tc.tile_pool(name="opool", bufs=2) as opool, \
         tc.tile_pool(name="psC", bufs=8, space="PSUM") as psC:

        zt = {}
        zdiff = {}

        def emit_z_mms(b):
            """z = H^T x + bias (bias rides the passthroughs)."""
            ztl = zpool.tile([P, 2, NZ], BF16, tag="zt", name=f"z_{b}")
            zt[b] = ztl
            xf = [xp[b][tr].rearrange("p a b -> p (a b)") for tr in range(2)]
            for m in range(2):
                for (off, ln) in ZBLK:
                    zps = psC.tile([P, 512], F32, tag="o", name=f"zp_{b}_{m}_{off}")
                    for tr in range(2):
                        nc.tensor.matmul(zps[:, :ln],
                                         H_sb[:, tr, m * P:(m + 1) * P],
                                         xf[tr][:, off:off + ln],
                                         start=(tr == 0), stop=(tr == 1))
                    nc.scalar.activation(ztl[:, m, off:off + ln], zps[:, :ln],
                                         ACTF.Identity, bias=bias_sb[:, m, :],
                                         scale=1.0)
            return ztl

        def emit_z_diffs(b):
            """The three z-difference arrays for stage 1."""
            ztl = zt[b]
            z3 = [ztl[:, m].rearrange("p (a b) -> p a b", b=PH) for m in range(2)]
            Dt = dpool.tile([P, 2, 66 * 65], BF16, tag="D", name=f"D_{b}")
            Dzt = dpool.tile([P, 2, NY], BF16, tag="Dz", name=f"Dz_{b}")
            Azt = dpool.tile([P, 2, NY], BF16, tag="Az", name=f"Az_{b}")
            D3 = []
            for m in range(2):
                d3 = Dt[:, m].rearrange("p (a b) -> p a b", b=65)
                D3.append(d3)
                nc.vector.tensor_sub(d3[:], z3[m][:, :, 0:65], z3[m][:, :, 1:66])
                dz3 = Dzt[:, m].rearrange("p (a b) -> p a b", b=65)
                nc.vector.tensor_sub(dz3[:], d3[:, 0:65, :], d3[:, 1:66, :])
                az3 = Azt[:, m].rearrange("p (a b) -> p a b", b=65)
                nc.vector.tensor_sub(az3[:], z3[m][:, 0:65, 1:66],
                                     z3[m][:, 1:66, 1:66])
            zdiff[b] = (Dt, Dzt, Azt, D3, z3)

        def emit_s1(b):
            """y = T10^T.Dz + Q1.Az + Q0.d(r'+1,c') + z(r'+1,c'+1), 65x65."""
            Dt, Dzt, Azt, D3, z3 = zdiff[b]
            ytl = dpool.tile([P, 2, NY], BF16, tag="yt", name=f"y_{b}")
            for m in range(2):
                for (r0, nr) in YROW:
                    ln = nr * 65
                    yps = psC.tile([P, 512], F32, tag="o",
                                   name=f"yp_{b}_{m}_{r0}")
                    # passthrough z(r'+1,c'+1) as an identity matmul FIRST
                    # (depends only on z), then the tr=0 difference terms,
                    # then tr=1: the PE starts each group after only half the
                    # (serial, DVE-bound) difference arrays exist.
                    nc.tensor.matmul(yps[:, :ln], ID16[:],
                                     z3[m][:, 1 + r0:1 + r0 + nr, 1:66],
                                     start=True, stop=False)
                    for tr in range(2):
                        for ti, (lhs, rhs) in enumerate((
                            (T10_sb, Dzt[:, tr, r0 * 65:r0 * 65 + ln]),
                            (PQ_sb[:, 1], Azt[:, tr, r0 * 65:r0 * 65 + ln]),
                            (PQ_sb[:, 0], D3[tr][:, 1 + r0:1 + r0 + nr, 0:65]),
                        )):
                            nc.tensor.matmul(yps[:, :ln],
                                             lhs[:, tr, m * P:(m + 1) * P],
                                             rhs, start=False,
                                             stop=(tr == 1 and ti == 2))
                    nc.scalar.copy(ytl[:, m, r0 * 65:r0 * 65 + ln],
                                   yps[:, :ln])
            return ytl

        def emit_s2(b, ytl):
            """out = T32^T.DD + Q3.Ay + Q2.D'(r,c) + y(r,c), 64x64."""
            y3 = [ytl[:, m].rearrange("p (a b) -> p a b", b=65) for m in range(2)]
            Dpt = dpool.tile([P, 2, 65 * 64], BF16, tag="Dp", name=f"Dp_{b}")
            DDt = dpool.tile([P, 2, NPIX], BF16, tag="DD", name=f"DD_{b}")
            Ayt = dpool.tile([P, 2, NPIX], BF16, tag="Ay", name=f"Ay_{b}")
            Dp3 = []
            for m in range(2):
                dp3 = Dpt[:, m].rearrange("p (a b) -> p a b", b=64)
                Dp3.append(dp3)
                nc.vector.tensor_sub(dp3[:], y3[m][:, :, 0:64], y3[m][:, :, 1:65])
                dd3 = DDt[:, m].rearrange("p (a b) -> p a b", b=64)
                nc.vector.tensor_sub(dd3[:], dp3[:, 0:64, :], dp3[:, 1:65, :])
                ay3 = Ayt[:, m].rearrange("p (a b) -> p a b", b=64)
                nc.vector.tensor_sub(ay3[:], y3[m][:, 0:64, 1:65],
                                     y3[m][:, 1:65, 1:65])
            for m in range(2):
                for pb in range(8):
                    r0 = pb * 8
                    ops = psC.tile([P, 512], F32, tag="o",
                                   name=f"op_{b}_{m}_{pb}")
                    nc.tensor.matmul(ops[:], ID16[:],
                                     y3[m][:, 1 + r0:9 + r0, 1:65],
                                     start=True, stop=False)
                    for tr in range(2):
                        for ti, (lhs, rhs) in enumerate((
                            (T32_sb, DDt[:, tr, r0 * 64:r0 * 64 + 512]),
                            (PQ_sb[:, 3], Ayt[:, tr, r0 * 64:r0 * 64 + 512]),
                            (PQ_sb[:, 2], Dp3[tr][:, 1 + r0:9 + r0, 0:64]),
                        )):
                            nc.tensor.matmul(ops[:],
                                             lhs[:, tr, m * P:(m + 1) * P],
                                             rhs, start=False,
                                             stop=(tr == 1 and ti == 2))
                    osb = opool.tile([P, 512], F32, tag="osb",
                                     name=f"osb_{b}_{m}_{pb}")
                    nc.scalar.copy(osb[:], ops[:])
                    nc.sync.dma_start(
                        out_ap[b, m * P:(m + 1) * P, r0:r0 + 8, :],
                        osb[:].rearrange("p (h w) -> p h w", w=H))

        emit_z_mms(0)
        emit_z_diffs(0)
        for b in range(B_CORE):
            ytl = emit_s1(b)
            if b + 1 < B_CORE:
                emit_z_mms(b + 1)
                emit_z_diffs(b + 1)
            emit_s2(b, ytl)


def build_program():
    from contextlib import ExitStack
    nc = bacc.Bacc("TRN2", target_bir_lowering=False, debug=False,
                   enable_asserts=False, num_devices=N_CORES)
    xs = nc.dram_tensor("xs", [B_CORE, C, H, H], F32, kind="ExternalInput").ap()
    pmk = nc.dram_tensor("pmk", [C, C], F32R, kind="ExternalInput").ap()
    u0k = nc.dram_tensor("u0k", [C, 1], F32R, kind="ExternalInput").ap()
    bias = nc.dram_tensor("bias", [C], F32, kind="ExternalInput").ap()
    out = nc.dram_tensor("out", [B_CORE, C, H, H], F32, kind="ExternalOutput").ap()
    with tile.TileContext(nc) as tc:
        with ExitStack() as ctx:
            build_body(tc, out, xs, pmk, u0k, bias, ctx)
    nc.compile()
    return nc


_cached_nc = None


def make_in_maps(x, pm, u0, b):
    in_maps = []
    for i in range(N_CORES):
        k = i if i < NK else i - NK
        in_maps.append({
            "xs": np.ascontiguousarray(x[i * B_CORE:(i + 1) * B_CORE]),
            "pmk": np.ascontiguousarray(pm[k]),
            "u0k": np.ascontiguousarray(u0[k]),
            "bias": np.ascontiguousarray(b),
        })
    return in_maps


def kernel(x, param_matrices, init_u, bias):
    global _cached_nc
    if _cached_nc is None:
        _cached_nc = build_program()
    nc = _cached_nc
    x = np.ascontiguousarray(np.asarray(x, dtype=np.float32))
    pm = np.ascontiguousarray(np.asarray(param_matrices, dtype=np.float32))
    u0 = np.ascontiguousarray(np.asarray(init_u, dtype=np.float32))
    b = np.ascontiguousarray(np.asarray(bias, dtype=np.float32))
    in_maps = make_in_maps(x, pm, u0, b)
    res = run_bass_kernel_spmd(nc, in_maps, core_ids=list(range(N_CORES)))
    return np.concatenate([r["out"] for r in res.results], axis=0)


if __name__ == "__main__":
    import reference
    inputs = {k: np.asarray(v) for k, v in reference.setup_inputs().items()}
    out = kernel(**inputs)
    print(out.shape, out.dtype)
